# revision 1
# baseline (speedup 1.0000x reference)
"""DirSAGE GNN message-passing kernel for 8x Trainium2 NeuronCores.

Strategy (sharding_hint: 1D node partition by destination, replicated weights,
halo exchange):
  - Nodes are block-partitioned across 8 cores (6250 each, padded to 6272).
  - Full node features h live (replicated) in each core's HBM, bf16 node-major,
    in a "remapped" row space: node n -> row (n // NP_CORE) * NPAD + n % NP_CORE
    so per-shard AllGather blocks land contiguously.
  - Per layer, each core computes both directed mean-aggregations for ITS nodes
    only, using edge lists pre-sorted by aggregation target (host prep):
      * gathered edge-source rows via the SWDGE dma_gather custom instruction
        (int16 indices => the row table is split at HALF=32768 into lo/hi)
      * segment-sum via one-hot matmuls accumulated in PSUM per 128-node window
        (S[e, j] = (dst_local[e] == j), built by one DVE is_equal per window)
      * mean via multiply with a partition-replicated 1/deg vector.
  - Layer update: psum[fo, n] = Wself.T @ hT + Wstd'.T @ aggstdT + Wdts'.T @ aggdtsT
    (alpha folded into W on host), then fused relu+bias on ACT -> bf16 hT.
  - New shard is PE-transposed to node-major, DMA'd to DRAM, AllGathered to
    rebuild the replicated h for the next layer's gathers.
  - JumpingKnowledge max is a running elementwise max; final Wlin matmul emits
    outT [64, nodes] fp32 per core; host concatenates and transposes.
"""

import math
import sys

sys.path.insert(0, "/opt/trn_rl_repo")

import numpy as np
import ml_dtypes

bf16 = ml_dtypes.bfloat16

# ----------------------------------------------------------------------------
# configuration
# ----------------------------------------------------------------------------

class CFG:
    N = 50000
    E = 800000
    D = 128
    OUT = 64
    L = 3
    NCORES = 8
    HALF = 25088          # gather-table split row (int16 limit; balanced halves)
    GW = 2                # windows per dma_gather buffer group
    SINGLE_PACKET = False # one descriptor packet per engine per gather call
    MAXC = 0              # if >0, split gather calls into <=MAXC-chunk subcalls
    NVALID = 0            # trailing -1 pads + per-core valid-count registers

    def __init__(self, **kw):
        for k, v in kw.items():
            setattr(self, k, v)
        self.NP_CORE = self.N // self.NCORES
        self.W = (self.NP_CORE + 127) // 128
        self.NPAD = self.W * 128
        self.NTOT = self.NPAD * self.NCORES
        assert self.HALF <= 32768 and self.NTOT - self.HALF <= 32768
        self.T512 = (self.NP_CORE + 511) // 512


# ----------------------------------------------------------------------------
# host-side preprocessing
# ----------------------------------------------------------------------------

def _wrap16(a):
    """idx layout for dma_gather: idx i -> partition i%16, slot i//16,
    replicated across the 8 groups of 16 partitions."""
    t = a.reshape(-1, 16).T.astype(np.int16)  # [16, n/16]
    return np.ascontiguousarray(np.tile(t, (8, 1)))  # [128, n/16]


def _prep_direction(cfg, agg_local, gather_remap):
    """agg_local: per-edge aggregation-target node, local [0, NP_CORE).
    gather_remap: per-edge gather-source row in remapped [0, NTOT) space.
    Returns per-window (lo_idx, lo_dst, hi_idx, hi_dst) lists (unpadded)."""
    order = np.argsort(agg_local, kind="stable")
    agg_local = agg_local[order]
    gather_remap = gather_remap[order]
    win = agg_local >> 7
    wins = []
    for w in range(cfg.W):
        m = win == w
        g = gather_remap[m]
        d = agg_local[m] - 128 * w
        lo = g < cfg.HALF
        wins.append((g[lo], d[lo], g[~lo] - cfg.HALF, d[~lo]))
    return wins


def _pad(a, tot, fill):
    out = np.full(tot, fill, np.int32)
    out[: len(a)] = a
    return out


def prep_inputs(cfg, x, edge_index, Wself, bself, Wstd, bstd, Wdts, bdts,
                Wlin, blin, alpha):
    """Returns (in_maps, consts) for the 8 cores."""
    N, NP, NPAD, NTOT, W = cfg.N, cfg.NP_CORE, cfg.NPAD, cfg.NTOT, cfg.W
    a = float(np.asarray(alpha).reshape(-1)[0])
    src = np.asarray(edge_index[0])
    dst = np.asarray(edge_index[1])

    deg_dst_r = 1.0 / np.clip(np.bincount(dst, minlength=N), 1, None).astype(np.float32)
    deg_src_r = 1.0 / np.clip(np.bincount(src, minlength=N), 1, None).astype(np.float32)

    remap = lambda n: (n // NP) * NPAD + (n % NP)

    # per-core, per-direction window groups
    per_core = []
    for p in range(cfg.NCORES):
        lo, hi = NP * p, NP * (p + 1)
        m_std = (dst >= lo) & (dst < hi)
        m_dts = (src >= lo) & (src < hi)
        wins_std = _prep_direction(cfg, dst[m_std] - lo, remap(src[m_std]))
        wins_dts = _prep_direction(cfg, src[m_dts] - lo, remap(dst[m_dts]))
        per_core.append((wins_std, wins_dts))

    # global chunk budgets (compile-time constants, same for every core)
    def budget(di, half):
        mx = 1
        for std, dts in per_core:
            for wtup in (std if di == 0 else dts):
                mx = max(mx, (len(wtup[2 * half]) + 127) // 128)
        return mx

    budgets = ((budget(0, 0), budget(0, 1)), (budget(1, 0), budget(1, 1)))

    # folded weights / biases (shared across cores)
    L, D, OUT = cfg.L, cfg.D, cfg.OUT
    wmats = np.concatenate(
        [np.stack([Wself[l], (1 - a) * Wstd[l], a * Wdts[l]]) for l in range(L)]
    ).astype(bf16)                                            # [3L, D, D]
    # sbuf layout [fi, m*D + fo] so each [D, D] slice is an lhsT
    wmats = np.ascontiguousarray(np.transpose(wmats, (1, 0, 2)).reshape(D, 3 * L * D))
    bias = np.stack(
        [bself[l] + (1 - a) * bstd[l] + a * bdts[l] for l in range(L)]
    ).astype(np.float32).T.copy()                              # [D, L]
    wlin = np.asarray(Wlin).astype(bf16)                       # [D, OUT]
    blin_c = np.asarray(blin).astype(np.float32).reshape(OUT, 1).copy()

    # padded-remapped x (gather source, replicated)
    xpad = np.zeros((NTOT, D), bf16)
    xv = np.asarray(x)
    for p in range(cfg.NCORES):
        xpad[p * NPAD : p * NPAD + NP] = xv[p * NP : (p + 1) * NP].astype(bf16)

    iota = np.arange(128, dtype=np.float32).astype(bf16).reshape(1, 128)
    ident = np.eye(128, dtype=np.float32).astype(bf16)

    in_maps = []
    for p in range(cfg.NCORES):
        im = {
            "xpad": xpad,
            "wmats": wmats,
            "wlin": wlin,
            "bias": bias,
            "blin": blin_c,
            "iota": iota,
            "ident": ident,
        }
        xT = np.zeros((D, NPAD), bf16)
        xT[:, :NP] = xv[p * NP : (p + 1) * NP].T.astype(bf16)
        im["xT"] = xT
        for di, dname in enumerate(("std", "dts")):
            wins = per_core[p][di]
            CL, CH = budgets[di]
            C = CL + CH
            ilo_parts, ihi_parts, dl_cols = [], [], []
            nv = []
            fill = -1 if cfg.NVALID else 0
            for w in range(W):
                g_lo, d_lo, g_hi, d_hi = wins[w]
                if cfg.NVALID and len(g_lo) == 0:
                    g_lo = np.zeros(1, np.int32)
                if cfg.NVALID and len(g_hi) == 0:
                    g_hi = np.zeros(1, np.int32)
                nv += [len(g_lo), len(g_hi)]
                ilo_parts.append(_pad(g_lo, CL * 128, fill))
                ihi_parts.append(_pad(g_hi, CH * 128, fill))
                dl_w = np.concatenate(
                    [_pad(d_lo, CL * 128, 255), _pad(d_hi, CH * 128, 255)]
                )
                dl_cols.append(dl_w.reshape(C, 128).T)  # [128, C]
            im[f"ilo_{dname}"] = _wrap16(np.concatenate(ilo_parts))
            im[f"ihi_{dname}"] = _wrap16(np.concatenate(ihi_parts))
            im[f"dl_{dname}"] = np.ascontiguousarray(
                np.concatenate(dl_cols, axis=1).astype(bf16)
            )  # [128, W*C]
            dr = deg_dst_r if di == 0 else deg_src_r
            dpad = np.ones((1, NPAD), np.float32)
            dpad[0, :NP] = dr[p * NP : (p + 1) * NP]
            im[f"degr_{dname}"] = dpad.astype(bf16)
            im[f"nv_{dname}"] = np.asarray(nv, np.int32).reshape(1, 2 * W)
        in_maps.append(im)

    return in_maps, budgets


# ----------------------------------------------------------------------------
# device program
# ----------------------------------------------------------------------------

def build_program(cfg, budgets):
    import concourse.bacc as bacc
    import concourse.bass as bass
    import concourse.mybir as mybir
    import concourse.tile as tile

    f32 = mybir.dt.float32
    b16 = mybir.dt.bfloat16
    i16 = mybir.dt.int16
    EQ = mybir.AluOpType.is_equal
    MULT = mybir.AluOpType.mult
    MAX = mybir.AluOpType.max

    N, NP, NPAD, NTOT, W, D, OUT, L = (
        cfg.N, cfg.NP_CORE, cfg.NPAD, cfg.NTOT, cfg.W, cfg.D, cfg.OUT, cfg.L,
    )
    HALF, GW = cfg.HALF, cfg.GW

    nc = bacc.Bacc("TRN2", target_bir_lowering=False, debug=False,
                   enable_asserts=False, num_devices=cfg.NCORES)

    # dram I/O
    xpad_d = nc.dram_tensor("xpad", [NTOT, D], b16, kind="ExternalInput")
    xT_d = nc.dram_tensor("xT", [D, NPAD], b16, kind="ExternalInput")
    wmats_d = nc.dram_tensor("wmats", [D, 3 * L * D], b16, kind="ExternalInput")
    wlin_d = nc.dram_tensor("wlin", [D, OUT], b16, kind="ExternalInput")
    bias_d = nc.dram_tensor("bias", [D, L], f32, kind="ExternalInput")
    blin_d = nc.dram_tensor("blin", [OUT, 1], f32, kind="ExternalInput")
    iota_d = nc.dram_tensor("iota", [1, 128], b16, kind="ExternalInput")
    ident_d = nc.dram_tensor("ident", [128, 128], b16, kind="ExternalInput")
    idx_d, dl_d, degr_d = {}, {}, {}
    for di, dname in enumerate(("std", "dts")):
        CL, CH = budgets[di]
        idx_d[dname] = (
            nc.dram_tensor(f"ilo_{dname}", [128, W * CL * 8], i16, kind="ExternalInput"),
            nc.dram_tensor(f"ihi_{dname}", [128, W * CH * 8], i16, kind="ExternalInput"),
        )
        dl_d[dname] = nc.dram_tensor(f"dl_{dname}", [128, W * (CL + CH)], b16,
                                     kind="ExternalInput")
        idx_d[dname] += (nc.dram_tensor(f"nv_{dname}", [1, 2 * W], mybir.dt.int32,
                                        kind="ExternalInput"),)
        degr_d[dname] = nc.dram_tensor(f"degr_{dname}", [1, NPAD], b16,
                                       kind="ExternalInput")
    outT_d = nc.dram_tensor("outT", [OUT, NPAD], f32, kind="ExternalOutput")

    with tile.TileContext(nc) as tc, \
         tc.tile_pool(name="resident", bufs=1) as rpool, \
         tc.tile_pool(name="rdram", bufs=1, space="DRAM") as dpool:
        def mktile(shape, dt, name, space=None, addr_space="Local"):
            pool = dpool if space == "DRAM" else rpool
            return pool.tile(shape, dt, name=name, tag=name, addr_space=addr_space)

        # dram internal tiles for halo exchange
        h_shard = mktile([NPAD, D], b16, "h_shard", space="DRAM")
        hbuf = [
            mktile([NTOT, D], b16, f"hbuf{i}", space="DRAM", addr_space="Shared")
            for i in range(L - 1)
        ]

        # resident sbuf tiles
        hT = [mktile([D, NPAD], b16, f"hT{i}") for i in range(2)]
        hmaxT = mktile([D, NPAD], b16, "hmaxT")
        aggT = {n: mktile([D, NPAD], b16, f"agg_{n}") for n in ("std", "dts")}
        staging = mktile([128, W, D], b16, "staging")
        wmats_s = mktile([D, 3 * L * D], b16, "wmats_s")
        wlin_s = mktile([D, OUT], b16, "wlin_s")
        bias_s = mktile([D, L], f32, "bias_s")
        blin_s = mktile([OUT, 1], f32, "blin_s")
        iota_s = mktile([128, 128], b16, "iota_s")
        ident_s = mktile([128, 128], b16, "ident_s")
        idx_s, dl_s, degr_s = {}, {}, {}
        for di, dname in enumerate(("std", "dts")):
            CL, CH = budgets[di]
            idx_s[dname] = (
                mktile([128, W * CL * 8], i16, f"ilo_s_{dname}"),
                mktile([128, W * CH * 8], i16, f"ihi_s_{dname}"),
            )
            dl_s[dname] = mktile([128, W * (CL + CH)], b16, f"dl_s_{dname}")
            idx_s[dname] += (mktile([1, 2 * W], mybir.dt.int32, f"nv_s_{dname}"),)
            degr_s[dname] = mktile([128, NPAD], b16, f"degr_s_{dname}")

        # constant loads
        nc.sync.dma_start(out=hT[0][:], in_=xT_d[:])
        nc.sync.dma_start(out=wmats_s[:], in_=wmats_d[:])
        nc.sync.dma_start(out=wlin_s[:], in_=wlin_d[:])
        nc.sync.dma_start(out=bias_s[:], in_=bias_d[:])
        nc.sync.dma_start(out=blin_s[:], in_=blin_d[:])
        nc.sync.dma_start(out=iota_s[:], in_=iota_d[:].to_broadcast([128, 128]))
        nc.sync.dma_start(out=ident_s[:], in_=ident_d[:])
        for dname in ("std", "dts"):
            nc.sync.dma_start(out=idx_s[dname][0][:], in_=idx_d[dname][0][:])
            nc.sync.dma_start(out=idx_s[dname][1][:], in_=idx_d[dname][1][:])
            nc.sync.dma_start(out=idx_s[dname][2][:], in_=idx_d[dname][2][:])
            nc.sync.dma_start(out=dl_s[dname][:], in_=dl_d[dname][:])
            nc.sync.dma_start(out=degr_s[dname][:],
                              in_=degr_d[dname][:].to_broadcast([128, NPAD]))
        # zero pad-tail of the ping-pong hT (transpose reads the full NPAD)
        if NPAD > NP:
            nc.vector.memset(hT[1][:, NP:], 0.0)

        with (
            tc.tile_pool(name="gpool", bufs=2) as gpool,
            tc.tile_pool(name="spool", bufs=2) as spool,
            tc.tile_pool(name="opool", bufs=2) as opool,
            tc.tile_pool(name="psag", bufs=4, space="PSUM") as psag,
            tc.tile_pool(name="pslayer", bufs=2, space="PSUM") as pslayer,
            tc.tile_pool(name="pstr", bufs=2, space="PSUM") as pstr,
        ):
            if cfg.NVALID:
                for di in range(2):
                    CL, CH = budgets[di]
                    for _ in range(2):
                        t1 = gpool.tile([128, cfg.GW * CL, D], b16, tag=f"glo{di}")
                        nc.vector.memset(t1[:], 0.0)
                        t2 = gpool.tile([128, cfg.GW * CH, D], b16, tag=f"ghi{di}")
                        nc.vector.memset(t2[:], 0.0)
            for layer in range(L):
                cur, nxt = hT[layer % 2], hT[(layer + 1) % 2]
                hsrc = xpad_d if layer == 0 else hbuf[layer - 1]
                src_lo = hsrc[0:HALF, :]
                src_hi = hsrc[HALF:NTOT, :]

                for di, dname in enumerate(("std", "dts")):
                    CL, CH = budgets[di]
                    C = CL + CH
                    ilo, ihi, nvs = idx_s[dname]
                    dl = dl_s[dname]
                    for wg in range(0, W, GW):
                        gwn = min(GW, W - wg)
                        glo = gpool.tile([128, GW * CL, D], b16, tag=f"glo{di}")
                        ghi = gpool.tile([128, GW * CH, D], b16, tag=f"ghi{di}")
                        def emit_gathers(gbuf, src_ap, itab, nch, base_ch, nvi):
                            # base_ch in per-direction chunk units of this half
                            step = cfg.MAXC if cfg.MAXC > 0 else nch
                            for s0 in range(0, nch, step):
                                sn = min(step, nch - s0)
                                if cfg.NVALID:
                                    cnt = nc.values_load(
                                        nvs[0:1, nvi : nvi + 1],
                                        engines=(mybir.EngineType.Pool,),
                                        skip_runtime_bounds_check=True,
                                    )
                                else:
                                    cnt = sn * 128
                                nc.gpsimd.dma_gather(
                                    gbuf[:, s0 : s0 + sn, :], src_ap,
                                    itab[:, (base_ch + s0) * 8 : (base_ch + s0 + sn) * 8],
                                    sn * 128, cnt, D,
                                    single_packet=bool(cfg.SINGLE_PACKET),
                                )
                        emit_gathers(glo, src_lo, ilo, gwn * CL, wg * CL, 2 * wg)
                        emit_gathers(ghi, src_hi, ihi, gwn * CH, wg * CH, 2 * wg + 1)
                        for j in range(gwn):
                            w = wg + j
                            S = spool.tile([128, C, 128], b16, tag="S")
                            nc.vector.tensor_tensor(
                                out=S[:],
                                in0=dl[:, w * C : (w + 1) * C]
                                .unsqueeze(2).to_broadcast([128, C, 128]),
                                in1=iota_s[:].unsqueeze(1).to_broadcast([128, C, 128]),
                                op=EQ,
                            )
                            ps = psag.tile([128, 128], f32)
                            for c in range(C):
                                G = (glo[:, j * CL + c, :] if c < CL
                                     else ghi[:, j * CH + (c - CL), :])
                                nc.tensor.matmul(
                                    ps[:], lhsT=G, rhs=S[:, c, :],
                                    start=(c == 0), stop=(c == C - 1),
                                )
                            nco = min(128, NP - 128 * w)
                            nc.vector.tensor_tensor(
                                out=aggT[dname][:, 128 * w : 128 * w + nco],
                                in0=ps[:, :nco],
                                in1=degr_s[dname][:, 128 * w : 128 * w + nco],
                                op=MULT,
                            )

                # layer update
                for t in range(cfg.T512):
                    a0, b0 = 512 * t, min(512 * (t + 1), NP)
                    n = b0 - a0
                    ps2 = pslayer.tile([128, 512], f32)
                    for k, rhs in enumerate((cur, aggT["std"], aggT["dts"])):
                        nc.tensor.matmul(
                            ps2[:, :n],
                            lhsT=wmats_s[:, (3 * layer + k) * D : (3 * layer + k + 1) * D],
                            rhs=rhs[:, a0:b0],
                            start=(k == 0), stop=(k == 2),
                        )
                    nc.scalar.activation(
                        out=nxt[:, a0:b0], in_=ps2[:, :n],
                        func=mybir.ActivationFunctionType.Relu,
                        bias=bias_s[:, layer : layer + 1], scale=1.0,
                    )

                # running JK max
                if layer == 0:
                    nc.vector.tensor_copy(out=hmaxT[:, :NP], in_=nxt[:, :NP])
                else:
                    nc.vector.tensor_tensor(
                        out=hmaxT[:, :NP], in0=hmaxT[:, :NP], in1=nxt[:, :NP], op=MAX
                    )

                # write-back + halo exchange (not needed after last layer)
                if layer < L - 1:
                    for t in range(W):
                        pt = pstr.tile([128, 128], b16)
                        nc.tensor.transpose(
                            out=pt[:], in_=nxt[:, 128 * t : 128 * (t + 1)],
                            identity=ident_s[:],
                        )
                        nc.scalar.copy(out=staging[:, t, :], in_=pt[:])
                    nc.sync.dma_start(
                        out=h_shard[:].rearrange("(t p) f -> p t f", p=128),
                        in_=staging[:],
                    )
                    nc.gpsimd.collective_compute(
                        "AllGather",
                        mybir.AluOpType.bypass,
                        replica_groups=[list(range(cfg.NCORES))],
                        ins=[h_shard[:]],
                        outs=[hbuf[layer][:]],
                    )

            # final linear layer -> outT [OUT, NPAD] fp32
            for t in range(cfg.T512):
                a0, b0 = 512 * t, min(512 * (t + 1), NP)
                n = b0 - a0
                ps2 = pslayer.tile([128, 512], f32)
                nc.tensor.matmul(
                    ps2[:OUT, :n], lhsT=wlin_s[:], rhs=hmaxT[:, a0:b0],
                    start=True, stop=True,
                )
                ot = opool.tile([OUT, 512], f32, tag="ot")
                nc.scalar.activation(
                    out=ot[:, :n], in_=ps2[:OUT, :n],
                    func=mybir.ActivationFunctionType.Identity,
                    bias=blin_s[:, 0:1], scale=1.0,
                )
                nc.sync.dma_start(out=outT_d[:, a0:b0], in_=ot[:, :n])

    nc.compile()
    return nc


# ----------------------------------------------------------------------------
# entry point
# ----------------------------------------------------------------------------

_CACHE = {}


def run(cfg, inputs, profile=False):
    from concourse.bass_utils import run_bass_kernel_spmd

    in_maps, budgets = prep_inputs(cfg, **inputs)
    key = (cfg.N, cfg.E, budgets)
    if key not in _CACHE:
        _CACHE[key] = build_program(cfg, budgets)
    nc = _CACHE[key]
    res = run_bass_kernel_spmd(
        nc, in_maps, core_ids=list(range(cfg.NCORES)), trace=profile
    )
    NP = cfg.NP_CORE
    out = np.concatenate(
        [res.results[p]["outT"][:, :NP].T for p in range(cfg.NCORES)], axis=0
    ).astype(np.float32)
    return out, res


def kernel(**inputs):
    cfg = CFG()
    out, _ = run(cfg, inputs, profile=False)
    return out



# revision 5
# speedup vs baseline: 2.2582x; 2.2582x over previous
"""DirSAGE GNN message-passing kernel for 8x Trainium2 NeuronCores.

Strategy (sharding_hint: 1D node partition by destination, replicated weights,
halo exchange):
  - Nodes are block-partitioned across 8 cores (6250 each, padded to 6272).
  - Full node features h live (replicated) in each core's HBM, bf16 node-major,
    in a "remapped" row space: node n -> row (n // NP_CORE) * NPAD + n % NP_CORE
    so per-shard AllGather blocks land contiguously.
  - Per layer, each core computes both directed mean-aggregations for ITS nodes
    only, using edge lists pre-sorted by aggregation target (host prep):
      * gathered edge-source rows via the SWDGE dma_gather custom instruction
        (int16 indices => the row table is split at HALF=32768 into lo/hi)
      * segment-sum via one-hot matmuls accumulated in PSUM per 128-node window
        (S[e, j] = (dst_local[e] == j), built by one DVE is_equal per window)
      * mean via multiply with a partition-replicated 1/deg vector.
  - Layer update: psum[fo, n] = Wself.T @ hT + Wstd'.T @ aggstdT + Wdts'.T @ aggdtsT
    (alpha folded into W on host), then fused relu+bias on ACT -> bf16 hT.
  - New shard is PE-transposed to node-major, DMA'd to DRAM, AllGathered to
    rebuild the replicated h for the next layer's gathers.
  - JumpingKnowledge max is a running elementwise max; final Wlin matmul emits
    outT [64, nodes] fp32 per core; host concatenates and transposes.
"""

import math
import sys

sys.path.insert(0, "/opt/trn_rl_repo")

import numpy as np
import ml_dtypes

bf16 = ml_dtypes.bfloat16

# ----------------------------------------------------------------------------
# configuration
# ----------------------------------------------------------------------------

class CFG:
    N = 50000
    E = 800000
    D = 128
    OUT = 64
    L = 3
    NCORES = 8
    HALF = 25088          # gather-table split row (int16 limit; balanced halves)
    GW = 2                # windows per dma_gather buffer group
    SINGLE_PACKET = False # one descriptor packet per engine per gather call
    MAXC = 0              # if >0, split gather calls into <=MAXC-chunk subcalls
    NVALID = 0            # trailing -1 pads + per-core valid-count registers
    NQ = 4                # SWDGE queues; queue q runs descgen on Q7 cores 2q,2q+1

    def __init__(self, **kw):
        for k, v in kw.items():
            setattr(self, k, v)
        self.NP_CORE = self.N // self.NCORES
        self.W = (self.NP_CORE + 127) // 128
        self.NPAD = self.W * 128
        self.NTOT = self.NPAD * self.NCORES
        assert self.HALF <= 32768 and self.NTOT - self.HALF <= 32768
        self.T512 = (self.NP_CORE + 511) // 512


# ----------------------------------------------------------------------------
# host-side preprocessing
# ----------------------------------------------------------------------------

def _wrap16(a):
    """idx layout for dma_gather: idx i -> partition i%16, slot i//16,
    replicated across the 8 groups of 16 partitions."""
    t = a.reshape(-1, 16).T.astype(np.int16)  # [16, n/16]
    return np.ascontiguousarray(np.tile(t, (8, 1)))  # [128, n/16]


def _prep_direction(cfg, agg_local, gather_remap):
    """agg_local: per-edge aggregation-target node, local [0, NP_CORE).
    gather_remap: per-edge gather-source row in remapped [0, NTOT) space.
    Returns per-window (lo_idx, lo_dst, hi_idx, hi_dst) lists (unpadded)."""
    order = np.argsort(agg_local, kind="stable")
    agg_local = agg_local[order]
    gather_remap = gather_remap[order]
    win = agg_local >> 7
    wins = []
    for w in range(cfg.W):
        m = win == w
        g = gather_remap[m]
        d = agg_local[m] - 128 * w
        lo = g < cfg.HALF
        wins.append((g[lo], d[lo], g[~lo] - cfg.HALF, d[~lo]))
    return wins


def _pad(a, tot, fill):
    out = np.full(tot, fill, np.int32)
    out[: len(a)] = a
    return out


def prep_inputs(cfg, x, edge_index, Wself, bself, Wstd, bstd, Wdts, bdts,
                Wlin, blin, alpha):
    """Returns (in_maps, consts) for the 8 cores."""
    N, NP, NPAD, NTOT, W = cfg.N, cfg.NP_CORE, cfg.NPAD, cfg.NTOT, cfg.W
    a = float(np.asarray(alpha).reshape(-1)[0])
    src = np.asarray(edge_index[0])
    dst = np.asarray(edge_index[1])

    deg_dst_r = 1.0 / np.clip(np.bincount(dst, minlength=N), 1, None).astype(np.float32)
    deg_src_r = 1.0 / np.clip(np.bincount(src, minlength=N), 1, None).astype(np.float32)

    remap = lambda n: (n // NP) * NPAD + (n % NP)

    # per-core, per-direction window groups
    per_core = []
    for p in range(cfg.NCORES):
        lo, hi = NP * p, NP * (p + 1)
        m_std = (dst >= lo) & (dst < hi)
        m_dts = (src >= lo) & (src < hi)
        wins_std = _prep_direction(cfg, dst[m_std] - lo, remap(src[m_std]))
        wins_dts = _prep_direction(cfg, src[m_dts] - lo, remap(dst[m_dts]))
        per_core.append((wins_std, wins_dts))

    # global chunk budgets (compile-time constants, same for every core)
    def budget(di, half):
        mx = 1
        for std, dts in per_core:
            for wtup in (std if di == 0 else dts):
                mx = max(mx, (len(wtup[2 * half]) + 127) // 128)
        return mx

    budgets = ((budget(0, 0), budget(0, 1)), (budget(1, 0), budget(1, 1)))

    # folded weights / biases (shared across cores)
    L, D, OUT = cfg.L, cfg.D, cfg.OUT
    wmats = np.concatenate(
        [np.stack([Wself[l], (1 - a) * Wstd[l], a * Wdts[l]]) for l in range(L)]
    ).astype(bf16)                                            # [3L, D, D]
    # sbuf layout [fi, m*D + fo] so each [D, D] slice is an lhsT
    wmats = np.ascontiguousarray(np.transpose(wmats, (1, 0, 2)).reshape(D, 3 * L * D))
    bias = np.stack(
        [bself[l] + (1 - a) * bstd[l] + a * bdts[l] for l in range(L)]
    ).astype(np.float32).T.copy()                              # [D, L]
    wlin = np.asarray(Wlin).astype(bf16)                       # [D, OUT]
    blin_c = np.asarray(blin).astype(np.float32).reshape(OUT, 1).copy()

    # padded-remapped x (gather source, replicated)
    xpad = np.zeros((NTOT, D), bf16)
    xv = np.asarray(x)
    for p in range(cfg.NCORES):
        xpad[p * NPAD : p * NPAD + NP] = xv[p * NP : (p + 1) * NP].astype(bf16)

    iota = np.arange(128, dtype=np.float32).astype(bf16).reshape(1, 128)
    ident = np.eye(128, dtype=np.float32).astype(bf16)

    in_maps = []
    for p in range(cfg.NCORES):
        im = {
            "xpad": xpad,
            "wmats": wmats,
            "wlin": wlin,
            "bias": bias,
            "blin": blin_c,
            "iota": iota,
            "ident": ident,
        }
        xT = np.zeros((D, NPAD), bf16)
        xT[:, :NP] = xv[p * NP : (p + 1) * NP].T.astype(bf16)
        im["xT"] = xT
        for di, dname in enumerate(("std", "dts")):
            wins = per_core[p][di]
            CL, CH = budgets[di]
            C = CL + CH
            ilo_parts, ihi_parts, dl_cols = [], [], []
            nv = []
            fill = -1 if cfg.NVALID else 0
            for w in range(W):
                g_lo, d_lo, g_hi, d_hi = wins[w]
                if cfg.NVALID and len(g_lo) == 0:
                    g_lo = np.zeros(1, np.int32)
                if cfg.NVALID and len(g_hi) == 0:
                    g_hi = np.zeros(1, np.int32)
                nv += [len(g_lo), len(g_hi)]
                ilo_parts.append(_pad(g_lo, CL * 128, fill))
                ihi_parts.append(_pad(g_hi, CH * 128, fill))
                dl_w = np.concatenate(
                    [_pad(d_lo, CL * 128, 255), _pad(d_hi, CH * 128, 255)]
                )
                dl_cols.append(dl_w.reshape(C, 128).T)  # [128, C]
            im[f"ilo_{dname}"] = _wrap16(np.concatenate(ilo_parts))
            im[f"ihi_{dname}"] = _wrap16(np.concatenate(ihi_parts))
            im[f"dl_{dname}"] = np.ascontiguousarray(
                np.concatenate(dl_cols, axis=1).astype(bf16)
            )  # [128, W*C]
            dr = deg_dst_r if di == 0 else deg_src_r
            dpad = np.ones((1, NPAD), np.float32)
            dpad[0, :NP] = dr[p * NP : (p + 1) * NP]
            im[f"degr_{dname}"] = dpad.astype(bf16)
            im[f"nv_{dname}"] = np.asarray(nv, np.int32).reshape(1, 2 * W)
        in_maps.append(im)

    return in_maps, budgets


# ----------------------------------------------------------------------------
# device program
# ----------------------------------------------------------------------------

def build_program(cfg, budgets):
    import concourse.bacc as bacc
    import concourse.bass as bass
    import concourse.mybir as mybir
    import concourse.tile as tile

    f32 = mybir.dt.float32
    b16 = mybir.dt.bfloat16
    i16 = mybir.dt.int16
    EQ = mybir.AluOpType.is_equal
    MULT = mybir.AluOpType.mult
    MAX = mybir.AluOpType.max

    N, NP, NPAD, NTOT, W, D, OUT, L = (
        cfg.N, cfg.NP_CORE, cfg.NPAD, cfg.NTOT, cfg.W, cfg.D, cfg.OUT, cfg.L,
    )
    HALF, GW = cfg.HALF, cfg.GW

    nc = bacc.Bacc("TRN2", target_bir_lowering=False, debug=False,
                   enable_asserts=False, num_devices=cfg.NCORES,
                   num_swdge_queues=cfg.NQ)

    # dram I/O
    xpad_d = nc.dram_tensor("xpad", [NTOT, D], b16, kind="ExternalInput")
    xT_d = nc.dram_tensor("xT", [D, NPAD], b16, kind="ExternalInput")
    wmats_d = nc.dram_tensor("wmats", [D, 3 * L * D], b16, kind="ExternalInput")
    wlin_d = nc.dram_tensor("wlin", [D, OUT], b16, kind="ExternalInput")
    bias_d = nc.dram_tensor("bias", [D, L], f32, kind="ExternalInput")
    blin_d = nc.dram_tensor("blin", [OUT, 1], f32, kind="ExternalInput")
    iota_d = nc.dram_tensor("iota", [1, 128], b16, kind="ExternalInput")
    ident_d = nc.dram_tensor("ident", [128, 128], b16, kind="ExternalInput")
    idx_d, dl_d, degr_d = {}, {}, {}
    for di, dname in enumerate(("std", "dts")):
        CL, CH = budgets[di]
        idx_d[dname] = (
            nc.dram_tensor(f"ilo_{dname}", [128, W * CL * 8], i16, kind="ExternalInput"),
            nc.dram_tensor(f"ihi_{dname}", [128, W * CH * 8], i16, kind="ExternalInput"),
        )
        dl_d[dname] = nc.dram_tensor(f"dl_{dname}", [128, W * (CL + CH)], b16,
                                     kind="ExternalInput")
        idx_d[dname] += (nc.dram_tensor(f"nv_{dname}", [1, 2 * W], mybir.dt.int32,
                                        kind="ExternalInput"),)
        degr_d[dname] = nc.dram_tensor(f"degr_{dname}", [1, NPAD], b16,
                                       kind="ExternalInput")
    outT_d = nc.dram_tensor("outT", [OUT, NPAD], f32, kind="ExternalOutput")

    with tile.TileContext(nc) as tc, \
         tc.tile_pool(name="resident", bufs=1) as rpool, \
         tc.tile_pool(name="rdram", bufs=1, space="DRAM") as dpool:
        def mktile(shape, dt, name, space=None, addr_space="Local"):
            pool = dpool if space == "DRAM" else rpool
            return pool.tile(shape, dt, name=name, tag=name, addr_space=addr_space)

        # dram internal tiles for halo exchange
        h_shard = mktile([NPAD, D], b16, "h_shard", space="DRAM")
        hbuf = [
            mktile([NTOT, D], b16, f"hbuf{i}", space="DRAM", addr_space="Shared")
            for i in range(L - 1)
        ]

        # resident sbuf tiles
        hT = [mktile([D, NPAD], b16, f"hT{i}") for i in range(2)]
        hmaxT = mktile([D, NPAD], b16, "hmaxT")
        aggT = {n: mktile([D, NPAD], b16, f"agg_{n}") for n in ("std", "dts")}
        staging = mktile([128, W, D], b16, "staging")
        wmats_s = mktile([D, 3 * L * D], b16, "wmats_s")
        wlin_s = mktile([D, OUT], b16, "wlin_s")
        bias_s = mktile([D, L], f32, "bias_s")
        blin_s = mktile([OUT, 1], f32, "blin_s")
        iota_s = mktile([128, 128], b16, "iota_s")
        ident_s = mktile([128, 128], b16, "ident_s")
        idx_s, dl_s, degr_s = {}, {}, {}
        for di, dname in enumerate(("std", "dts")):
            CL, CH = budgets[di]
            idx_s[dname] = (
                mktile([128, W * CL * 8], i16, f"ilo_s_{dname}"),
                mktile([128, W * CH * 8], i16, f"ihi_s_{dname}"),
            )
            dl_s[dname] = mktile([128, W * (CL + CH)], b16, f"dl_s_{dname}")
            idx_s[dname] += (mktile([1, 2 * W], mybir.dt.int32, f"nv_s_{dname}"),)
            degr_s[dname] = mktile([128, NPAD], b16, f"degr_s_{dname}")

        # constant loads
        nc.sync.dma_start(out=hT[0][:], in_=xT_d[:])
        nc.sync.dma_start(out=wmats_s[:], in_=wmats_d[:])
        nc.sync.dma_start(out=wlin_s[:], in_=wlin_d[:])
        nc.sync.dma_start(out=bias_s[:], in_=bias_d[:])
        nc.sync.dma_start(out=blin_s[:], in_=blin_d[:])
        nc.sync.dma_start(out=iota_s[:], in_=iota_d[:].to_broadcast([128, 128]))
        nc.sync.dma_start(out=ident_s[:], in_=ident_d[:])
        for dname in ("std", "dts"):
            nc.sync.dma_start(out=idx_s[dname][0][:], in_=idx_d[dname][0][:])
            nc.sync.dma_start(out=idx_s[dname][1][:], in_=idx_d[dname][1][:])
            nc.sync.dma_start(out=idx_s[dname][2][:], in_=idx_d[dname][2][:])
            nc.sync.dma_start(out=dl_s[dname][:], in_=dl_d[dname][:])
            nc.sync.dma_start(out=degr_s[dname][:],
                              in_=degr_d[dname][:].to_broadcast([128, NPAD]))
        # zero pad-tail of the ping-pong hT (transpose reads the full NPAD)
        if NPAD > NP:
            nc.vector.memset(hT[1][:, NP:], 0.0)

        with (
            tc.tile_pool(name="gpool", bufs=2) as gpool,
            tc.tile_pool(name="spool", bufs=2) as spool,
            tc.tile_pool(name="opool", bufs=2) as opool,
            tc.tile_pool(name="psag", bufs=4, space="PSUM") as psag,
            tc.tile_pool(name="pslayer", bufs=2, space="PSUM") as pslayer,
            tc.tile_pool(name="pstr", bufs=2, space="PSUM") as pstr,
        ):
            if cfg.NVALID:
                for di in range(2):
                    CL, CH = budgets[di]
                    for _ in range(2):
                        t1 = gpool.tile([128, cfg.GW * CL, D], b16, tag=f"glo{di}")
                        nc.vector.memset(t1[:], 0.0)
                        t2 = gpool.tile([128, cfg.GW * CH, D], b16, tag=f"ghi{di}")
                        nc.vector.memset(t2[:], 0.0)
            gq = [0]  # round-robin SWDGE queue so descgen overlaps across Q7 pairs
            for layer in range(L):
                cur, nxt = hT[layer % 2], hT[(layer + 1) % 2]
                hsrc = xpad_d if layer == 0 else hbuf[layer - 1]
                src_lo = hsrc[0:HALF, :]
                src_hi = hsrc[HALF:NTOT, :]

                for di, dname in enumerate(("std", "dts")):
                    CL, CH = budgets[di]
                    C = CL + CH
                    ilo, ihi, nvs = idx_s[dname]
                    dl = dl_s[dname]
                    for wg in range(0, W, GW):
                        gwn = min(GW, W - wg)
                        glo = gpool.tile([128, GW * CL, D], b16, tag=f"glo{di}")
                        ghi = gpool.tile([128, GW * CH, D], b16, tag=f"ghi{di}")
                        def emit_gathers(gbuf, src_ap, itab, nch, base_ch, nvi):
                            # base_ch in per-direction chunk units of this half
                            step = cfg.MAXC if cfg.MAXC > 0 else nch
                            for s0 in range(0, nch, step):
                                sn = min(step, nch - s0)
                                if cfg.NVALID:
                                    cnt = nc.values_load(
                                        nvs[0:1, nvi : nvi + 1],
                                        engines=(mybir.EngineType.Pool,),
                                        skip_runtime_bounds_check=True,
                                    )
                                else:
                                    cnt = sn * 128
                                nc.gpsimd.dma_gather(
                                    gbuf[:, s0 : s0 + sn, :], src_ap,
                                    itab[:, (base_ch + s0) * 8 : (base_ch + s0 + sn) * 8],
                                    sn * 128, cnt, D,
                                    single_packet=bool(cfg.SINGLE_PACKET),
                                    queue_num=gq[0] % cfg.NQ,
                                )
                                gq[0] += 1
                        emit_gathers(glo, src_lo, ilo, gwn * CL, wg * CL, 2 * wg)
                        emit_gathers(ghi, src_hi, ihi, gwn * CH, wg * CH, 2 * wg + 1)
                        for j in range(gwn):
                            w = wg + j
                            S = spool.tile([128, C, 128], b16, tag="S")
                            nc.vector.tensor_tensor(
                                out=S[:],
                                in0=dl[:, w * C : (w + 1) * C]
                                .unsqueeze(2).to_broadcast([128, C, 128]),
                                in1=iota_s[:].unsqueeze(1).to_broadcast([128, C, 128]),
                                op=EQ,
                            )
                            ps = psag.tile([128, 128], f32)
                            for c in range(C):
                                G = (glo[:, j * CL + c, :] if c < CL
                                     else ghi[:, j * CH + (c - CL), :])
                                nc.tensor.matmul(
                                    ps[:], lhsT=G, rhs=S[:, c, :],
                                    start=(c == 0), stop=(c == C - 1),
                                )
                            nco = min(128, NP - 128 * w)
                            nc.vector.tensor_tensor(
                                out=aggT[dname][:, 128 * w : 128 * w + nco],
                                in0=ps[:, :nco],
                                in1=degr_s[dname][:, 128 * w : 128 * w + nco],
                                op=MULT,
                            )

                # layer update
                for t in range(cfg.T512):
                    a0, b0 = 512 * t, min(512 * (t + 1), NP)
                    n = b0 - a0
                    ps2 = pslayer.tile([128, 512], f32)
                    for k, rhs in enumerate((cur, aggT["std"], aggT["dts"])):
                        nc.tensor.matmul(
                            ps2[:, :n],
                            lhsT=wmats_s[:, (3 * layer + k) * D : (3 * layer + k + 1) * D],
                            rhs=rhs[:, a0:b0],
                            start=(k == 0), stop=(k == 2),
                        )
                    nc.scalar.activation(
                        out=nxt[:, a0:b0], in_=ps2[:, :n],
                        func=mybir.ActivationFunctionType.Relu,
                        bias=bias_s[:, layer : layer + 1], scale=1.0,
                    )

                # running JK max
                if layer == 0:
                    nc.vector.tensor_copy(out=hmaxT[:, :NP], in_=nxt[:, :NP])
                else:
                    nc.vector.tensor_tensor(
                        out=hmaxT[:, :NP], in0=hmaxT[:, :NP], in1=nxt[:, :NP], op=MAX
                    )

                # write-back + halo exchange (not needed after last layer)
                if layer < L - 1:
                    for t in range(W):
                        pt = pstr.tile([128, 128], b16)
                        nc.tensor.transpose(
                            out=pt[:], in_=nxt[:, 128 * t : 128 * (t + 1)],
                            identity=ident_s[:],
                        )
                        nc.scalar.copy(out=staging[:, t, :], in_=pt[:])
                    nc.sync.dma_start(
                        out=h_shard[:].rearrange("(t p) f -> p t f", p=128),
                        in_=staging[:],
                    )
                    nc.gpsimd.collective_compute(
                        "AllGather",
                        mybir.AluOpType.bypass,
                        replica_groups=[list(range(cfg.NCORES))],
                        ins=[h_shard[:]],
                        outs=[hbuf[layer][:]],
                    )

            # final linear layer -> outT [OUT, NPAD] fp32
            for t in range(cfg.T512):
                a0, b0 = 512 * t, min(512 * (t + 1), NP)
                n = b0 - a0
                ps2 = pslayer.tile([128, 512], f32)
                nc.tensor.matmul(
                    ps2[:OUT, :n], lhsT=wlin_s[:], rhs=hmaxT[:, a0:b0],
                    start=True, stop=True,
                )
                ot = opool.tile([OUT, 512], f32, tag="ot")
                nc.scalar.activation(
                    out=ot[:, :n], in_=ps2[:OUT, :n],
                    func=mybir.ActivationFunctionType.Identity,
                    bias=blin_s[:, 0:1], scale=1.0,
                )
                nc.sync.dma_start(out=outT_d[:, a0:b0], in_=ot[:, :n])

    nc.compile()
    return nc


# ----------------------------------------------------------------------------
# entry point
# ----------------------------------------------------------------------------

_CACHE = {}


def run(cfg, inputs, profile=False):
    from concourse.bass_utils import run_bass_kernel_spmd

    in_maps, budgets = prep_inputs(cfg, **inputs)
    key = (cfg.N, cfg.E, budgets)
    if key not in _CACHE:
        _CACHE[key] = build_program(cfg, budgets)
    nc = _CACHE[key]
    res = run_bass_kernel_spmd(
        nc, in_maps, core_ids=list(range(cfg.NCORES)), trace=profile
    )
    NP = cfg.NP_CORE
    out = np.concatenate(
        [res.results[p]["outT"][:, :NP].T for p in range(cfg.NCORES)], axis=0
    ).astype(np.float32)
    return out, res


def kernel(**inputs):
    cfg = CFG()
    out, _ = run(cfg, inputs, profile=False)
    return out



# revision 11
# speedup vs baseline: 2.7369x; 1.2120x over previous
"""DirSAGE GNN message-passing kernel for 8x Trainium2 NeuronCores.

Strategy (sharding_hint: 1D node partition by destination, replicated weights,
halo exchange):
  - Nodes are block-partitioned across 8 cores (6250 each, padded to 6272).
  - Full node features h live (replicated) in each core's HBM, bf16 node-major,
    in a "remapped" row space: node n -> row (n // NP_CORE) * NPAD + n % NP_CORE
    so per-shard AllGather blocks land contiguously.
  - Per layer, each core computes both directed mean-aggregations for ITS nodes
    only, using edge lists pre-sorted by aggregation target (host prep):
      * gathered edge-source rows via the SWDGE dma_gather custom instruction
        (int16 indices => the row table is split at HALF=32768 into lo/hi)
      * segment-sum via one-hot matmuls accumulated in PSUM per 128-node window
        (S[e, j] = (dst_local[e] == j), built by one DVE is_equal per window)
      * mean via multiply with a partition-replicated 1/deg vector.
  - Layer update: psum[fo, n] = Wself.T @ hT + Wstd'.T @ aggstdT + Wdts'.T @ aggdtsT
    (alpha folded into W on host), then fused relu+bias on ACT -> bf16 hT.
  - New shard is PE-transposed to node-major, DMA'd to DRAM, AllGathered to
    rebuild the replicated h for the next layer's gathers.
  - JumpingKnowledge max is a running elementwise max; final Wlin matmul emits
    outT [64, nodes] fp32 per core; host concatenates and transposes.
"""

import math
import sys

sys.path.insert(0, "/opt/trn_rl_repo")

import numpy as np
import ml_dtypes

bf16 = ml_dtypes.bfloat16

# ----------------------------------------------------------------------------
# configuration
# ----------------------------------------------------------------------------

class CFG:
    N = 50000
    E = 800000
    D = 128
    OUT = 64
    L = 3
    NCORES = 8
    HALF = 25088          # gather-table split row (int16 limit; balanced halves)
    GW = 1                # windows per dma_gather buffer group
    GBUFS = 4             # gather tile ring depth (windows in flight)
    SINGLE_PACKET = False # one descriptor packet per engine per gather call
    MAXC = 0              # if >0, split gather calls into <=MAXC-chunk subcalls
    NVALID = 0            # trailing -1 pads + per-core valid-count registers
    NQ = 4                # SWDGE queues; queue q runs descgen on Q7 cores 2q,2q+1

    def __init__(self, **kw):
        for k, v in kw.items():
            setattr(self, k, v)
        self.NP_CORE = self.N // self.NCORES
        self.W = (self.NP_CORE + 127) // 128
        self.NPAD = self.W * 128
        self.NTOT = self.NPAD * self.NCORES
        assert self.HALF <= 32768 and self.NTOT - self.HALF <= 32768
        self.T512 = (self.NP_CORE + 511) // 512


# ----------------------------------------------------------------------------
# host-side preprocessing
# ----------------------------------------------------------------------------

def _wrap16(a):
    """idx layout for dma_gather: idx i -> partition i%16, slot i//16,
    replicated across the 8 groups of 16 partitions."""
    t = a.reshape(-1, 16).T.astype(np.int16)  # [16, n/16]
    return np.ascontiguousarray(np.tile(t, (8, 1)))  # [128, n/16]


def _prep_direction(cfg, agg_local, gather_remap):
    """agg_local: per-edge aggregation-target node, local [0, NP_CORE).
    gather_remap: per-edge gather-source row in remapped [0, NTOT) space.
    Returns per-window (lo_idx, lo_dst, hi_idx, hi_dst) lists (unpadded)."""
    order = np.argsort(agg_local, kind="stable")
    agg_local = agg_local[order]
    gather_remap = gather_remap[order]
    win = agg_local >> 7
    wins = []
    for w in range(cfg.W):
        m = win == w
        g = gather_remap[m]
        d = agg_local[m] - 128 * w
        lo = g < cfg.HALF
        wins.append((g[lo], d[lo], g[~lo] - cfg.HALF, d[~lo]))
    return wins


def _pad(a, tot, fill):
    out = np.full(tot, fill, np.int32)
    out[: len(a)] = a
    return out


def prep_inputs(cfg, x, edge_index, Wself, bself, Wstd, bstd, Wdts, bdts,
                Wlin, blin, alpha):
    """Returns (in_maps, consts) for the 8 cores."""
    N, NP, NPAD, NTOT, W = cfg.N, cfg.NP_CORE, cfg.NPAD, cfg.NTOT, cfg.W
    a = float(np.asarray(alpha).reshape(-1)[0])
    src = np.asarray(edge_index[0])
    dst = np.asarray(edge_index[1])

    deg_dst_r = 1.0 / np.clip(np.bincount(dst, minlength=N), 1, None).astype(np.float32)
    deg_src_r = 1.0 / np.clip(np.bincount(src, minlength=N), 1, None).astype(np.float32)

    remap = lambda n: (n // NP) * NPAD + (n % NP)

    # per-core, per-direction window groups
    per_core = []
    for p in range(cfg.NCORES):
        lo, hi = NP * p, NP * (p + 1)
        m_std = (dst >= lo) & (dst < hi)
        m_dts = (src >= lo) & (src < hi)
        wins_std = _prep_direction(cfg, dst[m_std] - lo, remap(src[m_std]))
        wins_dts = _prep_direction(cfg, src[m_dts] - lo, remap(dst[m_dts]))
        per_core.append((wins_std, wins_dts))

    # global chunk budgets (compile-time constants, same for every core)
    def budget(di, half):
        mx = 1
        for std, dts in per_core:
            for wtup in (std if di == 0 else dts):
                mx = max(mx, (len(wtup[2 * half]) + 127) // 128)
        return mx

    budgets = ((budget(0, 0), budget(0, 1)), (budget(1, 0), budget(1, 1)))

    # folded weights / biases (shared across cores)
    L, D, OUT = cfg.L, cfg.D, cfg.OUT
    wmats = np.concatenate(
        [np.stack([Wself[l], (1 - a) * Wstd[l], a * Wdts[l]]) for l in range(L)]
    ).astype(bf16)                                            # [3L, D, D]
    # sbuf layout [fi, m*D + fo] so each [D, D] slice is an lhsT
    wmats = np.ascontiguousarray(np.transpose(wmats, (1, 0, 2)).reshape(D, 3 * L * D))
    bias = np.stack(
        [bself[l] + (1 - a) * bstd[l] + a * bdts[l] for l in range(L)]
    ).astype(np.float32).T.copy()                              # [D, L]
    wlin = np.asarray(Wlin).astype(bf16)                       # [D, OUT]
    blin_c = np.asarray(blin).astype(np.float32).reshape(OUT, 1).copy()

    # padded-remapped x (gather source, replicated)
    xpad = np.zeros((NTOT, D), bf16)
    xv = np.asarray(x)
    for p in range(cfg.NCORES):
        xpad[p * NPAD : p * NPAD + NP] = xv[p * NP : (p + 1) * NP].astype(bf16)

    iota = np.arange(128, dtype=np.float32).astype(bf16).reshape(1, 128)
    ident = np.eye(128, dtype=np.float32).astype(bf16)

    in_maps = []
    for p in range(cfg.NCORES):
        im = {
            "xpad": xpad,
            "wmats": wmats,
            "wlin": wlin,
            "bias": bias,
            "blin": blin_c,
            "iota": iota,
            "ident": ident,
        }
        xT = np.zeros((D, NPAD), bf16)
        xT[:, :NP] = xv[p * NP : (p + 1) * NP].T.astype(bf16)
        im["xT"] = xT
        for di, dname in enumerate(("std", "dts")):
            wins = per_core[p][di]
            CL, CH = budgets[di]
            C = CL + CH
            ilo_parts, ihi_parts, dl_cols = [], [], []
            nv = []
            fill = -1 if cfg.NVALID else 0
            for w in range(W):
                g_lo, d_lo, g_hi, d_hi = wins[w]
                if cfg.NVALID and len(g_lo) == 0:
                    g_lo = np.zeros(1, np.int32)
                if cfg.NVALID and len(g_hi) == 0:
                    g_hi = np.zeros(1, np.int32)
                nv += [len(g_lo), len(g_hi)]
                ilo_parts.append(_pad(g_lo, CL * 128, fill))
                ihi_parts.append(_pad(g_hi, CH * 128, fill))
                dl_w = np.concatenate(
                    [_pad(d_lo, CL * 128, 255), _pad(d_hi, CH * 128, 255)]
                )
                dl_cols.append(dl_w.reshape(C, 128).T)  # [128, C]
            im[f"ilo_{dname}"] = _wrap16(np.concatenate(ilo_parts))
            im[f"ihi_{dname}"] = _wrap16(np.concatenate(ihi_parts))
            im[f"dl_{dname}"] = np.ascontiguousarray(
                np.concatenate(dl_cols, axis=1).astype(bf16)
            )  # [128, W*C]
            dr = deg_dst_r if di == 0 else deg_src_r
            dpad = np.ones((1, NPAD), np.float32)
            dpad[0, :NP] = dr[p * NP : (p + 1) * NP]
            im[f"degr_{dname}"] = dpad.astype(bf16)
            im[f"nv_{dname}"] = np.asarray(nv, np.int32).reshape(1, 2 * W)
        in_maps.append(im)

    return in_maps, budgets


# ----------------------------------------------------------------------------
# device program
# ----------------------------------------------------------------------------

def build_program(cfg, budgets):
    import concourse.bacc as bacc
    import concourse.bass as bass
    import concourse.mybir as mybir
    import concourse.tile as tile

    f32 = mybir.dt.float32
    b16 = mybir.dt.bfloat16
    i16 = mybir.dt.int16
    EQ = mybir.AluOpType.is_equal
    MULT = mybir.AluOpType.mult
    MAX = mybir.AluOpType.max

    N, NP, NPAD, NTOT, W, D, OUT, L = (
        cfg.N, cfg.NP_CORE, cfg.NPAD, cfg.NTOT, cfg.W, cfg.D, cfg.OUT, cfg.L,
    )
    HALF, GW = cfg.HALF, cfg.GW

    nc = bacc.Bacc("TRN2", target_bir_lowering=False, debug=False,
                   enable_asserts=False, num_devices=cfg.NCORES,
                   num_swdge_queues=cfg.NQ)

    # dram I/O
    xpad_d = nc.dram_tensor("xpad", [NTOT, D], b16, kind="ExternalInput")
    xT_d = nc.dram_tensor("xT", [D, NPAD], b16, kind="ExternalInput")
    wmats_d = nc.dram_tensor("wmats", [D, 3 * L * D], b16, kind="ExternalInput")
    wlin_d = nc.dram_tensor("wlin", [D, OUT], b16, kind="ExternalInput")
    bias_d = nc.dram_tensor("bias", [D, L], f32, kind="ExternalInput")
    blin_d = nc.dram_tensor("blin", [OUT, 1], f32, kind="ExternalInput")
    iota_d = nc.dram_tensor("iota", [1, 128], b16, kind="ExternalInput")
    ident_d = nc.dram_tensor("ident", [128, 128], b16, kind="ExternalInput")
    idx_d, dl_d, degr_d = {}, {}, {}
    for di, dname in enumerate(("std", "dts")):
        CL, CH = budgets[di]
        idx_d[dname] = (
            nc.dram_tensor(f"ilo_{dname}", [128, W * CL * 8], i16, kind="ExternalInput"),
            nc.dram_tensor(f"ihi_{dname}", [128, W * CH * 8], i16, kind="ExternalInput"),
        )
        dl_d[dname] = nc.dram_tensor(f"dl_{dname}", [128, W * (CL + CH)], b16,
                                     kind="ExternalInput")
        idx_d[dname] += (nc.dram_tensor(f"nv_{dname}", [1, 2 * W], mybir.dt.int32,
                                        kind="ExternalInput"),)
        degr_d[dname] = nc.dram_tensor(f"degr_{dname}", [1, NPAD], b16,
                                       kind="ExternalInput")
    outT_d = nc.dram_tensor("outT", [OUT, NPAD], f32, kind="ExternalOutput")

    with tile.TileContext(nc) as tc, \
         tc.tile_pool(name="resident", bufs=1) as rpool, \
         tc.tile_pool(name="rdram", bufs=1, space="DRAM") as dpool:
        def mktile(shape, dt, name, space=None, addr_space="Local"):
            pool = dpool if space == "DRAM" else rpool
            return pool.tile(shape, dt, name=name, tag=name, addr_space=addr_space)

        # dram internal tiles for halo exchange
        h_shard = mktile([NPAD, D], b16, "h_shard", space="DRAM")
        hbuf = [
            mktile([NTOT, D], b16, f"hbuf{i}", space="DRAM", addr_space="Shared")
            for i in range(L - 1)
        ]

        # resident sbuf tiles
        hT = [mktile([D, NPAD], b16, f"hT{i}") for i in range(2)]
        hmaxT = mktile([D, NPAD], b16, "hmaxT")
        aggT = {n: mktile([D, NPAD], b16, f"agg_{n}") for n in ("std", "dts")}
        wmats_s = mktile([D, 3 * L * D], b16, "wmats_s")
        wlin_s = mktile([D, OUT], b16, "wlin_s")
        bias_s = mktile([D, L], f32, "bias_s")
        blin_s = mktile([OUT, 1], f32, "blin_s")
        iota_s = mktile([128, 128], b16, "iota_s")
        ident_s = mktile([128, 128], b16, "ident_s")
        idx_s, dl_s, degr_s = {}, {}, {}
        for di, dname in enumerate(("std", "dts")):
            CL, CH = budgets[di]
            idx_s[dname] = (
                mktile([128, W * CL * 8], i16, f"ilo_s_{dname}"),
                mktile([128, W * CH * 8], i16, f"ihi_s_{dname}"),
            )
            dl_s[dname] = mktile([128, W * (CL + CH)], b16, f"dl_s_{dname}")
            idx_s[dname] += (mktile([1, 2 * W], mybir.dt.int32, f"nv_s_{dname}"),)
            degr_s[dname] = mktile([128, NPAD], b16, f"degr_s_{dname}")

        # constant loads
        nc.sync.dma_start(out=hT[0][:], in_=xT_d[:])
        nc.sync.dma_start(out=wmats_s[:], in_=wmats_d[:])
        nc.sync.dma_start(out=wlin_s[:], in_=wlin_d[:])
        nc.sync.dma_start(out=bias_s[:], in_=bias_d[:])
        nc.sync.dma_start(out=blin_s[:], in_=blin_d[:])
        nc.sync.dma_start(out=iota_s[:], in_=iota_d[:].to_broadcast([128, 128]))
        nc.sync.dma_start(out=ident_s[:], in_=ident_d[:])
        for dname in ("std", "dts"):
            nc.sync.dma_start(out=idx_s[dname][0][:], in_=idx_d[dname][0][:])
            nc.sync.dma_start(out=idx_s[dname][1][:], in_=idx_d[dname][1][:])
            nc.sync.dma_start(out=idx_s[dname][2][:], in_=idx_d[dname][2][:])
            nc.sync.dma_start(out=dl_s[dname][:], in_=dl_d[dname][:])
            nc.sync.dma_start(out=degr_s[dname][:],
                              in_=degr_d[dname][:].to_broadcast([128, NPAD]))
        # zero pad-tail of the ping-pong hT (transpose reads the full NPAD)
        if NPAD > NP:
            nc.vector.memset(hT[1][:, NP:], 0.0)

        with (
            tc.tile_pool(name="gpool", bufs=cfg.GBUFS) as gpool,
            tc.tile_pool(name="spool", bufs=3) as spool,
            tc.tile_pool(name="stpool", bufs=2) as stpool,
            tc.tile_pool(name="opool", bufs=2) as opool,
            tc.tile_pool(name="psag", bufs=2, space="PSUM") as psag,
            tc.tile_pool(name="pslayer", bufs=2, space="PSUM") as pslayer,
            tc.tile_pool(name="pstr", bufs=2, space="PSUM") as pstr,
        ):
            if cfg.NVALID:
                for di in range(2):
                    CL, CH = budgets[di]
                    for _ in range(cfg.GBUFS):
                        t1 = gpool.tile([128, CL, D], b16, tag=f"glo{di}")
                        nc.vector.memset(t1[:], 0.0)
                        t2 = gpool.tile([128, CH, D], b16, tag=f"ghi{di}")
                        nc.vector.memset(t2[:], 0.0)
            gq = [0]  # round-robin SWDGE queue so descgen overlaps across Q7 pairs
            for layer in range(L):
                cur, nxt = hT[layer % 2], hT[(layer + 1) % 2]
                hsrc = xpad_d if layer == 0 else hbuf[layer - 1]
                src_lo = hsrc[0:HALF, :]
                src_hi = hsrc[HALF:NTOT, :]

                def emit_block_update(t):
                    """Layer update + JK max (+ write-back or final linear) for
                    the 512-col block t; emitted as soon as both directions'
                    aggregations for its 4 windows are in aggT."""
                    a0, b0 = 512 * t, min(512 * (t + 1), NP)
                    n = b0 - a0
                    ps2 = pslayer.tile([128, 512], f32)
                    for k, rhs in enumerate((cur, aggT["std"], aggT["dts"])):
                        nc.tensor.matmul(
                            ps2[:, :n],
                            lhsT=wmats_s[:, (3 * layer + k) * D : (3 * layer + k + 1) * D],
                            rhs=rhs[:, a0:b0],
                            start=(k == 0), stop=(k == 2),
                        )
                    nc.scalar.activation(
                        out=nxt[:, a0:b0], in_=ps2[:, :n],
                        func=mybir.ActivationFunctionType.Relu,
                        bias=bias_s[:, layer : layer + 1], scale=1.0,
                    )
                    if layer == 0:
                        nc.vector.tensor_copy(out=hmaxT[:, a0:b0], in_=nxt[:, a0:b0])
                    else:
                        nc.vector.tensor_tensor(
                            out=hmaxT[:, a0:b0], in0=hmaxT[:, a0:b0],
                            in1=nxt[:, a0:b0], op=MAX,
                        )
                    if layer < L - 1:
                        # transpose to node-major and stream this block's rows out
                        wlo, whi = 4 * t, min(4 * t + 4, W)
                        nw = whi - wlo
                        st = stpool.tile([128, 4, D], b16, tag="staging")
                        for wi in range(wlo, whi):
                            pt = pstr.tile([128, 128], b16)
                            nc.tensor.transpose(
                                out=pt[:], in_=nxt[:, 128 * wi : 128 * (wi + 1)],
                                identity=ident_s[:],
                            )
                            nc.scalar.copy(out=st[:, wi - wlo, :], in_=pt[:])
                        nc.sync.dma_start(
                            out=h_shard[128 * wlo : 128 * whi, :]
                            .rearrange("(t p) f -> p t f", p=128),
                            in_=st[:, :nw, :],
                        )
                    else:
                        # JumpingKnowledge done for these cols: final linear
                        ps3 = pslayer.tile([128, 512], f32)
                        nc.tensor.matmul(
                            ps3[:OUT, :n], lhsT=wlin_s[:], rhs=hmaxT[:, a0:b0],
                            start=True, stop=True,
                        )
                        ot = opool.tile([OUT, 512], f32, tag="ot")
                        nc.scalar.activation(
                            out=ot[:, :n], in_=ps3[:OUT, :n],
                            func=mybir.ActivationFunctionType.Identity,
                            bias=blin_s[:, 0:1], scale=1.0,
                        )
                        nc.sync.dma_start(out=outT_d[:, a0:b0], in_=ot[:, :n])

                for di, dname in enumerate(("std", "dts")):
                    CL, CH = budgets[di]
                    C = CL + CH
                    ilo, ihi, nvs = idx_s[dname]
                    dl = dl_s[dname]
                    for w in range(W):
                        glo = gpool.tile([128, CL, D], b16, tag=f"glo{di}")
                        ghi = gpool.tile([128, CH, D], b16, tag=f"ghi{di}")
                        def emit_gather(gbuf, src_ap, itab, nch, base_ch):
                            nc.gpsimd.dma_gather(
                                gbuf[:, 0:nch, :], src_ap,
                                itab[:, base_ch * 8 : (base_ch + nch) * 8],
                                nch * 128, nch * 128, D,
                                single_packet=bool(cfg.SINGLE_PACKET),
                                queue_num=gq[0] % cfg.NQ,
                            )
                            gq[0] += 1
                        emit_gather(glo, src_lo, ilo, CL, w * CL)
                        emit_gather(ghi, src_hi, ihi, CH, w * CH)
                        S = spool.tile([128, C, 128], b16, tag="S")
                        nc.vector.tensor_tensor(
                            out=S[:],
                            in0=dl[:, w * C : (w + 1) * C]
                            .unsqueeze(2).to_broadcast([128, C, 128]),
                            in1=iota_s[:].unsqueeze(1).to_broadcast([128, C, 128]),
                            op=EQ,
                        )
                        ps = psag.tile([128, 128], f32)
                        for c in range(C):
                            G = (glo[:, c, :] if c < CL else ghi[:, c - CL, :])
                            nc.tensor.matmul(
                                ps[:], lhsT=G, rhs=S[:, c, :],
                                start=(c == 0), stop=(c == C - 1),
                            )
                        nco = min(128, NP - 128 * w)
                        nc.vector.tensor_tensor(
                            out=aggT[dname][:, 128 * w : 128 * w + nco],
                            in0=ps[:, :nco],
                            in1=degr_s[dname][:, 128 * w : 128 * w + nco],
                            op=MULT,
                        )
                        if di == 1 and (w + 1) % 4 == 0:
                            emit_block_update((w + 1) // 4 - 1)
                    if di == 1 and W % 4 != 0:
                        emit_block_update(cfg.T512 - 1)

                if layer < L - 1:
                    nc.gpsimd.collective_compute(
                        "AllGather",
                        mybir.AluOpType.bypass,
                        replica_groups=[list(range(cfg.NCORES))],
                        ins=[h_shard[:]],
                        outs=[hbuf[layer][:]],
                    )

    nc.compile()
    return nc


# ----------------------------------------------------------------------------
# entry point
# ----------------------------------------------------------------------------

_CACHE = {}


def run(cfg, inputs, profile=False):
    from concourse.bass_utils import run_bass_kernel_spmd

    in_maps, budgets = prep_inputs(cfg, **inputs)
    key = (cfg.N, cfg.E, budgets)
    if key not in _CACHE:
        _CACHE[key] = build_program(cfg, budgets)
    nc = _CACHE[key]
    res = run_bass_kernel_spmd(
        nc, in_maps, core_ids=list(range(cfg.NCORES)), trace=profile
    )
    NP = cfg.NP_CORE
    out = np.concatenate(
        [res.results[p]["outT"][:, :NP].T for p in range(cfg.NCORES)], axis=0
    ).astype(np.float32)
    return out, res


def kernel(**inputs):
    cfg = CFG()
    out, _ = run(cfg, inputs, profile=False)
    return out



# revision 13
# speedup vs baseline: 3.5207x; 1.2864x over previous
"""DirSAGE GNN message-passing kernel for 8x Trainium2 NeuronCores.

Strategy (sharding_hint: 1D node partition by destination, replicated weights,
halo exchange):
  - Nodes are block-partitioned across 8 cores (6250 each, padded to 6272).
  - Full node features h live (replicated) in each core's HBM, bf16 node-major,
    in a "remapped" row space: node n -> row (n // NP_CORE) * NPAD + n % NP_CORE
    so per-shard AllGather blocks land contiguously.
  - Per layer, each core computes both directed mean-aggregations for ITS nodes
    only, using edge lists pre-sorted by aggregation target (host prep):
      * gathered edge-source rows via the SWDGE dma_gather custom instruction
        (int16 indices => the row table is split at HALF=32768 into lo/hi)
      * segment-sum via one-hot matmuls accumulated in PSUM per 128-node window
        (S[e, j] = (dst_local[e] == j), built by one DVE is_equal per window)
      * mean via multiply with a partition-replicated 1/deg vector.
  - Layer update: psum[fo, n] = Wself.T @ hT + Wstd'.T @ aggstdT + Wdts'.T @ aggdtsT
    (alpha folded into W on host), then fused relu+bias on ACT -> bf16 hT.
  - New shard is PE-transposed to node-major, DMA'd to DRAM, AllGathered to
    rebuild the replicated h for the next layer's gathers.
  - JumpingKnowledge max is a running elementwise max; final Wlin matmul emits
    outT [64, nodes] fp32 per core; host concatenates and transposes.
"""

import math
import sys

sys.path.insert(0, "/opt/trn_rl_repo")

import numpy as np
import ml_dtypes

bf16 = ml_dtypes.bfloat16

# ----------------------------------------------------------------------------
# configuration
# ----------------------------------------------------------------------------

class CFG:
    N = 50000
    E = 800000
    D = 128
    OUT = 64
    L = 3
    NCORES = 8
    HALF = 25088          # gather-table split row (int16 limit; balanced halves)
    GW = 1                # windows per dma_gather buffer group
    GBUFS = 5             # gather tile ring depth (windows in flight)
    SINGLE_PACKET = False # one descriptor packet per engine per gather call
    MAXC = 0              # if >0, split gather calls into <=MAXC-chunk subcalls
    NVALID = 1            # -1 pads: ucode trims trailing pads from descgen
    NQ = 4                # SWDGE queues; queue q runs descgen on Q7 cores 2q,2q+1

    def __init__(self, **kw):
        for k, v in kw.items():
            setattr(self, k, v)
        self.NP_CORE = self.N // self.NCORES
        self.W = (self.NP_CORE + 127) // 128
        self.NPAD = self.W * 128
        self.NTOT = self.NPAD * self.NCORES
        assert self.HALF <= 32768 and self.NTOT - self.HALF <= 32768
        self.T512 = (self.NP_CORE + 511) // 512


# ----------------------------------------------------------------------------
# host-side preprocessing
# ----------------------------------------------------------------------------

def _wrap16(a):
    """idx layout for dma_gather: idx i -> partition i%16, slot i//16,
    replicated across the 8 groups of 16 partitions."""
    t = a.reshape(-1, 16).T.astype(np.int16)  # [16, n/16]
    return np.ascontiguousarray(np.tile(t, (8, 1)))  # [128, n/16]


def _prep_direction(cfg, agg_local, gather_remap):
    """agg_local: per-edge aggregation-target node, local [0, NP_CORE).
    gather_remap: per-edge gather-source row in remapped [0, NTOT) space.
    Returns per-window (lo_idx, lo_dst, hi_idx, hi_dst) lists (unpadded)."""
    order = np.argsort(agg_local, kind="stable")
    agg_local = agg_local[order]
    gather_remap = gather_remap[order]
    win = agg_local >> 7
    wins = []
    for w in range(cfg.W):
        m = win == w
        g = gather_remap[m]
        d = agg_local[m] - 128 * w
        lo = g < cfg.HALF
        wins.append((g[lo], d[lo], g[~lo] - cfg.HALF, d[~lo]))
    return wins


def _pad(a, tot, fill):
    out = np.full(tot, fill, np.int32)
    out[: len(a)] = a
    return out


def prep_inputs(cfg, x, edge_index, Wself, bself, Wstd, bstd, Wdts, bdts,
                Wlin, blin, alpha):
    """Returns (in_maps, consts) for the 8 cores."""
    N, NP, NPAD, NTOT, W = cfg.N, cfg.NP_CORE, cfg.NPAD, cfg.NTOT, cfg.W
    a = float(np.asarray(alpha).reshape(-1)[0])
    src = np.asarray(edge_index[0])
    dst = np.asarray(edge_index[1])

    deg_dst_r = 1.0 / np.clip(np.bincount(dst, minlength=N), 1, None).astype(np.float32)
    deg_src_r = 1.0 / np.clip(np.bincount(src, minlength=N), 1, None).astype(np.float32)

    remap = lambda n: (n // NP) * NPAD + (n % NP)

    # per-core, per-direction window groups
    per_core = []
    for p in range(cfg.NCORES):
        lo, hi = NP * p, NP * (p + 1)
        m_std = (dst >= lo) & (dst < hi)
        m_dts = (src >= lo) & (src < hi)
        wins_std = _prep_direction(cfg, dst[m_std] - lo, remap(src[m_std]))
        wins_dts = _prep_direction(cfg, src[m_dts] - lo, remap(dst[m_dts]))
        per_core.append((wins_std, wins_dts))

    # global chunk budgets (compile-time constants, same for every core)
    def budget(di, half):
        mx = 1
        for std, dts in per_core:
            for wtup in (std if di == 0 else dts):
                mx = max(mx, (len(wtup[2 * half]) + 127) // 128)
        return mx

    budgets = ((budget(0, 0), budget(0, 1)), (budget(1, 0), budget(1, 1)))

    # folded weights / biases (shared across cores)
    L, D, OUT = cfg.L, cfg.D, cfg.OUT
    wmats = np.concatenate(
        [np.stack([Wself[l], (1 - a) * Wstd[l], a * Wdts[l]]) for l in range(L)]
    ).astype(bf16)                                            # [3L, D, D]
    # sbuf layout [fi, m*D + fo] so each [D, D] slice is an lhsT
    wmats = np.ascontiguousarray(np.transpose(wmats, (1, 0, 2)).reshape(D, 3 * L * D))
    bias = np.stack(
        [bself[l] + (1 - a) * bstd[l] + a * bdts[l] for l in range(L)]
    ).astype(np.float32).T.copy()                              # [D, L]
    wlin = np.asarray(Wlin).astype(bf16)                       # [D, OUT]
    blin_c = np.asarray(blin).astype(np.float32).reshape(OUT, 1).copy()

    # padded-remapped x (gather source, replicated)
    xpad = np.zeros((NTOT, D), bf16)
    xv = np.asarray(x)
    for p in range(cfg.NCORES):
        xpad[p * NPAD : p * NPAD + NP] = xv[p * NP : (p + 1) * NP].astype(bf16)

    iota = np.arange(128, dtype=np.float32).astype(bf16).reshape(1, 128)
    ident = np.eye(128, dtype=np.float32).astype(bf16)

    in_maps = []
    for p in range(cfg.NCORES):
        im = {
            "xpad": xpad,
            "wmats": wmats,
            "wlin": wlin,
            "bias": bias,
            "blin": blin_c,
            "iota": iota,
            "ident": ident,
        }
        xT = np.zeros((D, NPAD), bf16)
        xT[:, :NP] = xv[p * NP : (p + 1) * NP].T.astype(bf16)
        im["xT"] = xT
        for di, dname in enumerate(("std", "dts")):
            wins = per_core[p][di]
            CL, CH = budgets[di]
            C = CL + CH
            ilo_parts, ihi_parts, dl_cols = [], [], []
            nv = []
            fill = -1 if cfg.NVALID else 0
            for w in range(W):
                g_lo, d_lo, g_hi, d_hi = wins[w]
                if cfg.NVALID and len(g_lo) == 0:
                    g_lo = np.zeros(1, np.int32)
                if cfg.NVALID and len(g_hi) == 0:
                    g_hi = np.zeros(1, np.int32)
                nv += [len(g_lo), len(g_hi)]
                ilo_parts.append(_pad(g_lo, CL * 128, fill))
                ihi_parts.append(_pad(g_hi, CH * 128, fill))
                dl_w = np.concatenate(
                    [_pad(d_lo, CL * 128, 255), _pad(d_hi, CH * 128, 255)]
                )
                dl_cols.append(dl_w.reshape(C, 128).T)  # [128, C]
            im[f"ilo_{dname}"] = _wrap16(np.concatenate(ilo_parts))
            im[f"ihi_{dname}"] = _wrap16(np.concatenate(ihi_parts))
            im[f"dl_{dname}"] = np.ascontiguousarray(
                np.concatenate(dl_cols, axis=1).astype(bf16)
            )  # [128, W*C]
            dr = deg_dst_r if di == 0 else deg_src_r
            dpad = np.ones((1, NPAD), np.float32)
            dpad[0, :NP] = dr[p * NP : (p + 1) * NP]
            im[f"degr_{dname}"] = dpad.astype(bf16)
            im[f"nv_{dname}"] = np.asarray(nv, np.int32).reshape(1, 2 * W)
        in_maps.append(im)

    return in_maps, budgets


# ----------------------------------------------------------------------------
# device program
# ----------------------------------------------------------------------------

def build_program(cfg, budgets):
    import concourse.bacc as bacc
    import concourse.bass as bass
    import concourse.mybir as mybir
    import concourse.tile as tile

    f32 = mybir.dt.float32
    b16 = mybir.dt.bfloat16
    i16 = mybir.dt.int16
    EQ = mybir.AluOpType.is_equal
    MULT = mybir.AluOpType.mult
    MAX = mybir.AluOpType.max

    N, NP, NPAD, NTOT, W, D, OUT, L = (
        cfg.N, cfg.NP_CORE, cfg.NPAD, cfg.NTOT, cfg.W, cfg.D, cfg.OUT, cfg.L,
    )
    HALF, GW = cfg.HALF, cfg.GW

    nc = bacc.Bacc("TRN2", target_bir_lowering=False, debug=False,
                   enable_asserts=False, num_devices=cfg.NCORES,
                   num_swdge_queues=cfg.NQ)

    # dram I/O
    xpad_d = nc.dram_tensor("xpad", [NTOT, D], b16, kind="ExternalInput")
    xT_d = nc.dram_tensor("xT", [D, NPAD], b16, kind="ExternalInput")
    wmats_d = nc.dram_tensor("wmats", [D, 3 * L * D], b16, kind="ExternalInput")
    wlin_d = nc.dram_tensor("wlin", [D, OUT], b16, kind="ExternalInput")
    bias_d = nc.dram_tensor("bias", [D, L], f32, kind="ExternalInput")
    blin_d = nc.dram_tensor("blin", [OUT, 1], f32, kind="ExternalInput")
    iota_d = nc.dram_tensor("iota", [1, 128], b16, kind="ExternalInput")
    ident_d = nc.dram_tensor("ident", [128, 128], b16, kind="ExternalInput")
    idx_d, dl_d, degr_d = {}, {}, {}
    for di, dname in enumerate(("std", "dts")):
        CL, CH = budgets[di]
        idx_d[dname] = (
            nc.dram_tensor(f"ilo_{dname}", [128, W * CL * 8], i16, kind="ExternalInput"),
            nc.dram_tensor(f"ihi_{dname}", [128, W * CH * 8], i16, kind="ExternalInput"),
        )
        dl_d[dname] = nc.dram_tensor(f"dl_{dname}", [128, W * (CL + CH)], b16,
                                     kind="ExternalInput")
        idx_d[dname] += (nc.dram_tensor(f"nv_{dname}", [1, 2 * W], mybir.dt.int32,
                                        kind="ExternalInput"),)
        degr_d[dname] = nc.dram_tensor(f"degr_{dname}", [1, NPAD], b16,
                                       kind="ExternalInput")
    outT_d = nc.dram_tensor("outT", [OUT, NPAD], f32, kind="ExternalOutput")

    with tile.TileContext(nc) as tc, \
         tc.tile_pool(name="resident", bufs=1) as rpool, \
         tc.tile_pool(name="rdram", bufs=1, space="DRAM") as dpool:
        def mktile(shape, dt, name, space=None, addr_space="Local"):
            pool = dpool if space == "DRAM" else rpool
            return pool.tile(shape, dt, name=name, tag=name, addr_space=addr_space)

        # dram internal tiles for halo exchange
        h_shard = mktile([NPAD, D], b16, "h_shard", space="DRAM")
        hbuf = [
            mktile([NTOT, D], b16, f"hbuf{i}", space="DRAM", addr_space="Shared")
            for i in range(L - 1)
        ]

        # resident sbuf tiles
        hT = [mktile([D, NPAD], b16, f"hT{i}") for i in range(2)]
        hmaxT = mktile([D, NPAD], b16, "hmaxT")
        aggT = {n: mktile([D, NPAD], b16, f"agg_{n}") for n in ("std", "dts")}
        wmats_s = mktile([D, 3 * L * D], b16, "wmats_s")
        wlin_s = mktile([D, OUT], b16, "wlin_s")
        bias_s = mktile([D, L], f32, "bias_s")
        blin_s = mktile([OUT, 1], f32, "blin_s")
        iota_s = mktile([128, 128], b16, "iota_s")
        ident_s = mktile([128, 128], b16, "ident_s")
        idx_s, dl_s, degr_s = {}, {}, {}
        for di, dname in enumerate(("std", "dts")):
            CL, CH = budgets[di]
            idx_s[dname] = (
                mktile([128, W * CL * 8], i16, f"ilo_s_{dname}"),
                mktile([128, W * CH * 8], i16, f"ihi_s_{dname}"),
            )
            dl_s[dname] = mktile([128, W * (CL + CH)], b16, f"dl_s_{dname}")
            idx_s[dname] += (mktile([1, 2 * W], mybir.dt.int32, f"nv_s_{dname}"),)
            degr_s[dname] = mktile([128, NPAD], b16, f"degr_s_{dname}")

        # constant loads
        nc.sync.dma_start(out=hT[0][:], in_=xT_d[:])
        nc.sync.dma_start(out=wmats_s[:], in_=wmats_d[:])
        nc.sync.dma_start(out=wlin_s[:], in_=wlin_d[:])
        nc.sync.dma_start(out=bias_s[:], in_=bias_d[:])
        nc.sync.dma_start(out=blin_s[:], in_=blin_d[:])
        nc.sync.dma_start(out=iota_s[:], in_=iota_d[:].to_broadcast([128, 128]))
        nc.sync.dma_start(out=ident_s[:], in_=ident_d[:])
        for dname in ("std", "dts"):
            nc.sync.dma_start(out=idx_s[dname][0][:], in_=idx_d[dname][0][:])
            nc.sync.dma_start(out=idx_s[dname][1][:], in_=idx_d[dname][1][:])
            nc.sync.dma_start(out=idx_s[dname][2][:], in_=idx_d[dname][2][:])
            nc.sync.dma_start(out=dl_s[dname][:], in_=dl_d[dname][:])
            nc.sync.dma_start(out=degr_s[dname][:],
                              in_=degr_d[dname][:].to_broadcast([128, NPAD]))
        # zero pad-tail of the ping-pong hT (transpose reads the full NPAD)
        if NPAD > NP:
            nc.vector.memset(hT[1][:, NP:], 0.0)

        with (
            tc.tile_pool(name="gpool", bufs=cfg.GBUFS) as gpool,
            tc.tile_pool(name="spool", bufs=3) as spool,
            tc.tile_pool(name="stpool", bufs=2) as stpool,
            tc.tile_pool(name="opool", bufs=2) as opool,
            tc.tile_pool(name="psag", bufs=2, space="PSUM") as psag,
            tc.tile_pool(name="pslayer", bufs=2, space="PSUM") as pslayer,
            tc.tile_pool(name="pstr", bufs=2, space="PSUM") as pstr,
        ):
            if cfg.NVALID:
                for di in range(2):
                    CL, CH = budgets[di]
                    for _ in range(cfg.GBUFS):
                        t1 = gpool.tile([128, CL, D], b16, tag=f"glo{di}")
                        nc.vector.memset(t1[:], 0.0)
                        t2 = gpool.tile([128, CH, D], b16, tag=f"ghi{di}")
                        nc.vector.memset(t2[:], 0.0)
            gq = [0]  # round-robin SWDGE queue so descgen overlaps across Q7 pairs
            for layer in range(L):
                cur, nxt = hT[layer % 2], hT[(layer + 1) % 2]
                hsrc = xpad_d if layer == 0 else hbuf[layer - 1]
                src_lo = hsrc[0:HALF, :]
                src_hi = hsrc[HALF:NTOT, :]

                def emit_block_update(t):
                    """Layer update + JK max (+ write-back or final linear) for
                    the 512-col block t; emitted as soon as both directions'
                    aggregations for its 4 windows are in aggT."""
                    a0, b0 = 512 * t, min(512 * (t + 1), NP)
                    n = b0 - a0
                    ps2 = pslayer.tile([128, 512], f32)
                    for k, rhs in enumerate((cur, aggT["std"], aggT["dts"])):
                        nc.tensor.matmul(
                            ps2[:, :n],
                            lhsT=wmats_s[:, (3 * layer + k) * D : (3 * layer + k + 1) * D],
                            rhs=rhs[:, a0:b0],
                            start=(k == 0), stop=(k == 2),
                        )
                    nc.scalar.activation(
                        out=nxt[:, a0:b0], in_=ps2[:, :n],
                        func=mybir.ActivationFunctionType.Relu,
                        bias=bias_s[:, layer : layer + 1], scale=1.0,
                    )
                    if layer == 0:
                        nc.vector.tensor_copy(out=hmaxT[:, a0:b0], in_=nxt[:, a0:b0])
                    else:
                        nc.vector.tensor_tensor(
                            out=hmaxT[:, a0:b0], in0=hmaxT[:, a0:b0],
                            in1=nxt[:, a0:b0], op=MAX,
                        )
                    if layer < L - 1:
                        # transpose to node-major and stream this block's rows out
                        wlo, whi = 4 * t, min(4 * t + 4, W)
                        nw = whi - wlo
                        st = stpool.tile([128, 4, D], b16, tag="staging")
                        for wi in range(wlo, whi):
                            pt = pstr.tile([128, 128], b16)
                            nc.tensor.transpose(
                                out=pt[:], in_=nxt[:, 128 * wi : 128 * (wi + 1)],
                                identity=ident_s[:],
                            )
                            nc.scalar.copy(out=st[:, wi - wlo, :], in_=pt[:])
                        nc.sync.dma_start(
                            out=h_shard[128 * wlo : 128 * whi, :]
                            .rearrange("(t p) f -> p t f", p=128),
                            in_=st[:, :nw, :],
                        )
                    else:
                        # JumpingKnowledge done for these cols: final linear
                        ps3 = pslayer.tile([128, 512], f32)
                        nc.tensor.matmul(
                            ps3[:OUT, :n], lhsT=wlin_s[:], rhs=hmaxT[:, a0:b0],
                            start=True, stop=True,
                        )
                        ot = opool.tile([OUT, 512], f32, tag="ot")
                        nc.scalar.activation(
                            out=ot[:, :n], in_=ps3[:OUT, :n],
                            func=mybir.ActivationFunctionType.Identity,
                            bias=blin_s[:, 0:1], scale=1.0,
                        )
                        nc.sync.dma_start(out=outT_d[:, a0:b0], in_=ot[:, :n])

                for di, dname in enumerate(("std", "dts")):
                    CL, CH = budgets[di]
                    C = CL + CH
                    ilo, ihi, nvs = idx_s[dname]
                    dl = dl_s[dname]
                    for w in range(W):
                        glo = gpool.tile([128, CL, D], b16, tag=f"glo{di}")
                        ghi = gpool.tile([128, CH, D], b16, tag=f"ghi{di}")
                        def emit_gather(gbuf, src_ap, itab, nch, base_ch, nvi):
                            if cfg.NVALID:
                                # runtime valid count: decode's ring reservation
                                # must match the ucode's trailing(-1) trim
                                cnt = nc.values_load(
                                    nvs[0:1, nvi : nvi + 1],
                                    engines=(mybir.EngineType.Pool,),
                                    skip_runtime_bounds_check=True,
                                )
                            else:
                                cnt = nch * 128
                            nc.gpsimd.dma_gather(
                                gbuf[:, 0:nch, :], src_ap,
                                itab[:, base_ch * 8 : (base_ch + nch) * 8],
                                nch * 128, cnt, D,
                                single_packet=bool(cfg.SINGLE_PACKET),
                                queue_num=gq[0] % cfg.NQ,
                            )
                            gq[0] += 1
                        emit_gather(glo, src_lo, ilo, CL, w * CL, 2 * w)
                        emit_gather(ghi, src_hi, ihi, CH, w * CH, 2 * w + 1)
                        S = spool.tile([128, C, 128], b16, tag="S")
                        nc.vector.tensor_tensor(
                            out=S[:],
                            in0=dl[:, w * C : (w + 1) * C]
                            .unsqueeze(2).to_broadcast([128, C, 128]),
                            in1=iota_s[:].unsqueeze(1).to_broadcast([128, C, 128]),
                            op=EQ,
                        )
                        ps = psag.tile([128, 128], f32)
                        for c in range(C):
                            G = (glo[:, c, :] if c < CL else ghi[:, c - CL, :])
                            nc.tensor.matmul(
                                ps[:], lhsT=G, rhs=S[:, c, :],
                                start=(c == 0), stop=(c == C - 1),
                            )
                        nco = min(128, NP - 128 * w)
                        nc.vector.tensor_tensor(
                            out=aggT[dname][:, 128 * w : 128 * w + nco],
                            in0=ps[:, :nco],
                            in1=degr_s[dname][:, 128 * w : 128 * w + nco],
                            op=MULT,
                        )
                        if di == 1 and (w + 1) % 4 == 0:
                            emit_block_update((w + 1) // 4 - 1)
                    if di == 1 and W % 4 != 0:
                        emit_block_update(cfg.T512 - 1)

                if layer < L - 1:
                    nc.gpsimd.collective_compute(
                        "AllGather",
                        mybir.AluOpType.bypass,
                        replica_groups=[list(range(cfg.NCORES))],
                        ins=[h_shard[:]],
                        outs=[hbuf[layer][:]],
                    )

    nc.compile()
    return nc


# ----------------------------------------------------------------------------
# entry point
# ----------------------------------------------------------------------------

_CACHE = {}


def run(cfg, inputs, profile=False):
    from concourse.bass_utils import run_bass_kernel_spmd

    in_maps, budgets = prep_inputs(cfg, **inputs)
    key = (cfg.N, cfg.E, budgets)
    if key not in _CACHE:
        _CACHE[key] = build_program(cfg, budgets)
    nc = _CACHE[key]
    res = run_bass_kernel_spmd(
        nc, in_maps, core_ids=list(range(cfg.NCORES)), trace=profile
    )
    NP = cfg.NP_CORE
    out = np.concatenate(
        [res.results[p]["outT"][:, :NP].T for p in range(cfg.NCORES)], axis=0
    ).astype(np.float32)
    return out, res


def kernel(**inputs):
    cfg = CFG()
    out, _ = run(cfg, inputs, profile=False)
    return out

